# revision 8
# baseline (speedup 1.0000x reference)
"""Trainium2 Bass kernel for the Bayesian SNN problem.

Model (per reference):
  w1 = w1_mu + eps1 * exp(0.5*w1_logvar)          [2048, 4096]
  w2 = w2_mu + eps2 * exp(0.5*w2_logvar)          [4096, 1024]
  5-step LIF over batch 2048:
    mem = 0.95*mem + cur - (prev_mem > 1)
    spk = (mem > 1)
  out = sum_t spk2   -> [2048, 1024]

Strategy: pure data-parallel over batch (256 rows/core, 8 cores, no
collectives). The matmuls have no cross-timestep dependency, so each core
runs one batched GEMM over all (t, b) rows per layer; only the cheap
elementwise membrane scan is sequential in t.

GEMM1 runs as 2.13 passes instead of 3 bf16 passes: an fp16 main pass
(wh = fp16(w1), xh = fp16(x)) plus ONE fp8e4m3 DoubleRow correction pass
computing wl*xh + wh*xl with the residuals wl = w1-wh, xl = x-xh
pre-scaled by 2^11 so both products share one power-of-two psum scale
(main-pass weights also carry the 2^11 so everything accumulates in a
single psum group; the 2^-11 descale rides the psum-drain activation for
free). DoubleRow contracts 256 rows/instruction at fp8 rate, so the
correction pass costs ~1.13 K-passes -> per-tile matmul time drops from
25.6us to ~17.5us (HW-measured). Emulated numerics: 1879/2.1M output
flips, rel err 9.7e-3 (gate 2e-2); eps2 ships as fp16 (2012 flips), eps1
must stay f32 (fp16 eps1 -> 13k flips, over the gate).

GEMM2 is one fp16 pass split into two 512-wide output halves: half 0's
weights are generated directly into an SBUF tile that stays resident
through phase A (no DRAM round trip), half 1 goes through a DRAM fp16
scratch (a gpsimd cast-DMA, f32 SBUF -> f16 DRAM); the spike tensors
load once per timestep and stay resident across halves.

Spikes cross phases as fp8 (exact for {0,1}) in a t-major DRAM layout
[T, P, NT, BC] so phase B's loads are 256B-contiguous runs (the k-major
layout gathered 128B runs at ~140GB/s and fell behind the MM consume
rate). Phase A writes them per-(pair, timestep) so the A->B handoff only
waits on the LAST pair's t-slice, not a whole [P, TB] block; phase B
upcasts to fp16 on DVE (one [P, 8K] copy per timestep).

LIF in phase A runs on PAIRS of hidden tiles: the psum drains interleave
two tiles' currents into one [P, TB, 2] buffer so every LIF op covers 512
columns instead of 256 - halves the DVE op count, whose fixed ~0.3-0.4us
per-op cost otherwise backs up the strict-FIFO queues and (through the
lv/eps quarter-buffer WAR chain) stalls the w1-gen DMA stream.

Scheduling notes: an 80-MM dummy warmup holds the HAM clock-gate at K=8/8
through the pipeline fill (tiles 0/1 generate their weight splits at
quarter granularity so the first LDWEIGHTS can issue ~6us earlier); w1-gen
DMA issue is quarter-interleaved 2 tiles ahead with compute emitted after
the current tile's LIF (DVE/ACT queues are strict FIFO - order of emission
is everything); w2-gen is paced ~1.2 blocks per tile across tiles 1..26
with loads split scalar/gpsimd so no queue backs up behind the w1-gen
stream (sync); a 24-MM dummy chain at the top of phase B (kept live by an
ExternalOutput junk write - internal-scratch writes get dead-code
eliminated) bridges the pool-exit barrier so HAM never re-throttles at
the handoff.
"""

import ml_dtypes
import numpy as np

import concourse.bass as bass
import concourse.tile as tile
from concourse import bacc, mybir
from concourse.bass_utils import run_bass_kernel_spmd

F32 = mybir.dt.float32
BF16 = mybir.dt.bfloat16
F16 = mybir.dt.float16
F8 = mybir.dt.float8e4
ALU = mybir.AluOpType
ACTF = mybir.ActivationFunctionType
DR = mybir.MatmulPerfMode.DoubleRow

P = 128
B, T, DIN, DH, DOUT = 2048, 5, 2048, 4096, 1024
NCORES = 8
BC = B // NCORES            # 256 batch rows per core
TB = T * BC                 # 1280 batched-time rows per core
KO1 = DIN // P              # 16 contraction tiles, layer 1
NT = DH // P                # 32 hidden tiles
KO2 = DH // P               # 32 contraction tiles, layer 2
BETA = 0.95
THRESH = 1.0
WS = 2048.0                 # 2^11 residual / main-pass weight scale
DS = 1.0 / WS

MM_CHUNKS = ((0, 512), (512, 512), (1024, 256))


def _build_nc():
    nc = bacc.Bacc(
        "TRN2",
        target_bir_lowering=False,
        debug=False,
        num_devices=NCORES,
    )

    # x arrives host-split: fp16 main plane + fp8 e4m3 planes for the
    # DoubleRow correction (xh8 = e4m3(x), xl8 = e4m3((x - fp16(x))*2^11)),
    # all in [p, k_tile, tb] layout
    xh16_in = nc.dram_tensor("xh16", [P, KO1, TB], F16, kind="ExternalInput").ap()
    xh8_in = nc.dram_tensor("xh8", [P, KO1, TB], F8, kind="ExternalInput").ap()
    xl8_in = nc.dram_tensor("xl8", [P, KO1, TB], F8, kind="ExternalInput").ap()
    # w1 inputs come in host-pretransposed [p, n_tile, k_tile, col] layout:
    # each partition's (n) tile slice is 8KB contiguous -> full-rate DMA
    w1_mu = nc.dram_tensor("w1_mu", [P, NT, KO1, P], F32, kind="ExternalInput").ap()
    w1_lv = nc.dram_tensor("w1_logvar", [P, NT, KO1, P], F32, kind="ExternalInput").ap()
    eps1 = nc.dram_tensor("eps1", [P, NT, KO1, P], F32, kind="ExternalInput").ap()
    w2_mu = nc.dram_tensor("w2_mu", [DH, DOUT], F32, kind="ExternalInput").ap()
    w2_lv = nc.dram_tensor("w2_logvar", [DH, DOUT], F32, kind="ExternalInput").ap()
    eps2 = nc.dram_tensor("eps2", [DH, DOUT], F16, kind="ExternalInput").ap()
    out = nc.dram_tensor("out", [BC, DOUT], F32, kind="ExternalOutput").ap()
    # tiny real output keeping the warm-keeper chain alive through DCE
    junk = nc.dram_tensor("junk", [P, 8], BF16, kind="ExternalOutput").ap()

    with tile.TileContext(nc) as tc:
        with tc.tile_pool(name="dram", bufs=1, space="DRAM") as dramp:
            # t-major fp8 spikes: phase B's per-t load is 256B-contiguous
            spk1d = dramp.tile([T, P, NT, BC], F8)
            w2h2d = dramp.tile([KO2, P, 512], F16)

            # W2 half-0 residency and the warmup tiles live OUTSIDE the
            # phase scopes: w2-gen writes W2F1 during phase A and phase B
            # reads it; the warm tiles serve both the startup warmup and
            # the A->B handoff keeper.
            with (
                tc.tile_pool(name="w2res1", bufs=1) as w2r1,
                tc.tile_pool(name="warm", bufs=1) as wmp,
                tc.tile_pool(name="warmps", bufs=1, space="PSUM") as wmps,
            ):
                W2F1 = w2r1.tile([P, KO2, 512], F16)
                wma = wmp.tile([P, 128], BF16, tag="wma")
                wmb = wmp.tile([P, 512], BF16, tag="wmb")

                # ---------------- Phase A: layer 1 ----------------
                with (
                    tc.tile_pool(name="xres", bufs=1) as xp,
                    tc.tile_pool(name="aw1t", bufs=3) as wp1,
                    tc.tile_pool(name="aw1hl", bufs=3) as whl,
                    tc.tile_pool(name="aq", bufs=5) as qp,
                    tc.tile_pool(name="acur", bufs=1) as curp,
                    tc.tile_pool(name="aspk", bufs=1) as sp,
                    tc.tile_pool(name="amem", bufs=1) as mp,
                    tc.tile_pool(name="w2gen", bufs=1) as w2p,
                    tc.tile_pool(name="apsum", bufs=2, space="PSUM") as aps,
                ):
                    pend_data = {}

                    def emit_w1gen_loads(n):
                        """DMA mu/logvar/eps column-block for hidden tile n,
                        quarter-interleaved so the gen chain can start on
                        quarter 0 without waiting for the whole block."""
                        w1t = wp1.tile([P, KO1, P], F32, tag="w1t")
                        qs = []
                        for q in range(4):
                            osl = slice(q * 4, (q + 1) * 4)
                            lvq = qp.tile([P, 4, P], F32, tag="lv1")
                            epq = qp.tile([P, 4, P], F32, tag="ep1")
                            nc.sync.dma_start(lvq, w1_lv[:, n, osl, :])
                            nc.sync.dma_start(epq, eps1[:, n, osl, :])
                            nc.gpsimd.dma_start(w1t[:, osl, :], w1_mu[:, n, osl, :])
                            qs.append((lvq, epq))
                        pend_data[n] = (w1t, qs)

                    def emit_w1gen_compute(n, fine=False):
                        """Build w1 = mu + eps*exp(0.5*lv) in f32, then split
                        into the fp16 main-pass weights (x 2^11) and the fp8
                        DoubleRow pair (wl*2^11 interleaved with wh).
                        Emitted a full tile after its loads so the DVE/ACT
                        queue heads never sit blocked on the gen DMAs.
                        fine=True (tiles 0/1) also splits the weight
                        conversions per quarter so the first matmuls'
                        LDWEIGHTS can issue before the whole tile is in."""
                        w1t, qs = pend_data.pop(n)
                        wh16 = whl.tile([P, KO1, P], F16, tag="wh16")
                        w18i = whl.tile([P, 2, KO1, P], F8, tag="w18i")

                        def convert(osl):
                            # flatten contiguous [P, a, P] -> [P, a*128]: 3D
                            # APs overflow the TT sync-wait encoding
                            w1f = w1t[:, osl, :].rearrange("p a b -> p (a b)")
                            wh16f = wh16[:, osl, :].rearrange("p a b -> p (a b)")
                            wl8f = w18i[:, 0, osl, :].rearrange("p a b -> p (a b)")
                            wh8f = w18i[:, 1, osl, :].rearrange("p a b -> p (a b)")
                            # wh16 = fp16(w1)*2^11 (exact power-of-2 scale)
                            nc.scalar.activation(wh16f, w1f, ACTF.Copy, scale=WS)
                            # wl8 = e4m3(w1*2^11 - wh16)  (= residual * 2^11)
                            nc.vector.scalar_tensor_tensor(
                                wl8f, w1f, WS, wh16f, op0=ALU.mult, op1=ALU.subtract
                            )
                            # wh8 = e4m3(w1), unscaled (pairs with xl8*2^11)
                            nc.scalar.activation(wh8f, w1f, ACTF.Copy)

                        for q, (lvq, epq) in enumerate(qs):
                            osl = slice(q * 4, (q + 1) * 4)
                            lvf = lvq.rearrange("p a b -> p (a b)")
                            epf = epq.rearrange("p a b -> p (a b)")
                            w1f = w1t[:, osl, :].rearrange("p a b -> p (a b)")
                            nc.scalar.activation(lvf, lvf, ACTF.Exp, scale=0.5)
                            nc.vector.scalar_tensor_tensor(
                                epf, epf, 1.0, lvf, op0=ALU.bypass, op1=ALU.mult
                            )
                            # split the adds across DVE/gpsimd (gpsimd runs
                            # these ~3x slower than spec; don't chain 4 there)
                            if q < 2:
                                nc.vector.scalar_tensor_tensor(
                                    w1f, w1f, 1.0, epf, op0=ALU.bypass, op1=ALU.add
                                )
                            else:
                                nc.gpsimd.tensor_tensor(w1f, w1f, epf, op=ALU.add)
                            if fine:
                                convert(osl)
                        if not fine:
                            convert(slice(0, KO1))
                        return wh16, w18i

                    # w2-gen pacing: one full k-row block per emission,
                    # ~1.2 per tile across tiles 1..26 so the gpsimd/DMA
                    # tail fully drains before the A->B handoff
                    w2tile = [1 + (o2 * 26) // KO2 for o2 in range(KO2)]

                    def emit_w2gen(o2):
                        """One k-row block of w2 -> fp16: cols 0:512 straight
                        into the resident W2F1 tile, cols 512: via DRAM
                        scratch (gpsimd cast-DMA). Loads split scalar/gpsimd
                        so neither queue backs up behind the w1-gen stream
                        (sync)."""
                        orows = slice(o2 * P, (o2 + 1) * P)
                        mu2t = w2p.tile([P, DOUT], F32, tag="mu2")
                        lv2t = w2p.tile([P, DOUT], F32, tag="lv2")
                        ep2t = w2p.tile([P, DOUT], F16, tag="ep2")
                        nc.gpsimd.dma_start(lv2t, w2_lv[orows, :])
                        nc.gpsimd.dma_start(ep2t, eps2[orows, :])
                        nc.scalar.dma_start(mu2t, w2_mu[orows, :])
                        nc.scalar.activation(lv2t, lv2t, ACTF.Exp, scale=0.5)
                        nc.vector.scalar_tensor_tensor(
                            lv2t, lv2t, 1.0, ep2t, op0=ALU.bypass, op1=ALU.mult
                        )
                        nc.gpsimd.tensor_tensor(mu2t, mu2t, lv2t, op=ALU.add)
                        nc.scalar.activation(W2F1[:, o2, :], mu2t[:, 0:512], ACTF.Copy)
                        nc.gpsimd.dma_start(w2h2d[o2], mu2t[:, 512:DOUT])

                    # PE warmup: ~17us of dummy matmuls so the HAM clock gate
                    # releases (K=8/8) before the real stream starts; otherwise
                    # the first ~10 tiles run cold and re-throttle in a cascade
                    nc.vector.memset(wma, 0.0)
                    nc.vector.memset(wmb, 0.0)
                    wps = wmps.tile([P, 512], F32)
                    for i in range(80):
                        nc.tensor.matmul(wps, wma, wmb, start=(i == 0), stop=(i == 79))
                    nc.vector.tensor_copy(wmb[:, 0:256], wps[:, 0:256])

                    # w-gen for the first two tiles comes first so the ACT/DVE
                    # queues aren't head-of-line blocked behind the x staging;
                    # the x chunks interleave with the two fine gen-computes
                    # (scheduler runs ready DMAs while converts wait on loads)
                    XH16 = xp.tile([P, KO1, TB], F16)
                    X8I = xp.tile([P, 2, KO1, TB], F8)
                    emit_w1gen_loads(0)
                    emit_w1gen_loads(1)
                    pend = {0: emit_w1gen_compute(0, fine=True)}
                    nc.scalar.dma_start(XH16[:, 0:2, :], xh16_in[:, 0:2, :])
                    nc.scalar.dma_start(XH16[:, 2:4, :], xh16_in[:, 2:4, :])
                    pend[1] = emit_w1gen_compute(1, fine=True)
                    for c in range(2, 8):
                        osl = slice(c * 2, (c + 1) * 2)
                        nc.scalar.dma_start(XH16[:, osl, :], xh16_in[:, osl, :])
                    for c in range(4):
                        osl = slice(c * 4, (c + 1) * 4)
                        nc.gpsimd.dma_start(X8I[:, 0, osl, :], xh8_in[:, osl, :])
                        nc.gpsimd.dma_start(X8I[:, 1, osl, :], xl8_in[:, osl, :])

                    pcur = None
                    for n in range(NT):
                        wh16, w18i = pend.pop(n)
                        if n + 2 < NT:
                            emit_w1gen_loads(n + 2)

                        ps = aps.tile([P, TB], F32, tag="ps1")
                        # fp16 main pass (psum holds 2^11 * cur1)
                        for k in range(KO1):
                            for c0, cw in MM_CHUNKS:
                                nc.tensor.matmul(
                                    ps[:, c0 : c0 + cw],
                                    wh16[:, k, :],
                                    XH16[:, k, c0 : c0 + cw],
                                    start=(k == 0),
                                    stop=False,
                                )
                        # fp8 DoubleRow correction pass: per k-tile one MM
                        # contracts wl*xh (slot 0) + wh*xl (slot 1)
                        for k in range(KO1):
                            for c0, cw in MM_CHUNKS:
                                nc.tensor.matmul(
                                    ps[:, c0 : c0 + cw],
                                    w18i[:, :, k, :],
                                    X8I[:, :, k, c0 : c0 + cw],
                                    start=False,
                                    stop=(k == KO1 - 1),
                                    perf_mode=DR,
                                )

                        # drain (with the 2^-11 descale) scatters this
                        # tile's currents into its parity plane of the pair
                        # buffer [P, T, 2, BC] (t-major so every LIF slice
                        # AND every spike-DMA slice is contiguous); the psum
                        # recycles fast and the serial LIF chain below never
                        # gates the tensor engine
                        if n % 2 == 0:
                            pcur = curp.tile([P, T, 2, BC], F32, tag="cur1")
                        nc.scalar.activation(
                            pcur[:, :, n % 2, :], ps, ACTF.Copy, scale=DS
                        )

                        if n % 2 == 1:
                            # pair-LIF over both tiles' currents: every op is
                            # [P, 512] instead of [P, 256] - half the DVE ops
                            spkp = sp.tile([P, T, 2, BC], F8, tag="spk")
                            mem = mp.tile([P, 2, BC], F32, tag="mem1")
                            memf = mem.rearrange("p a b -> p (a b)")

                            def tsl(t):
                                return pcur[:, t, :, :].rearrange(
                                    "p a b -> p (a b)"
                                )

                            def ssl(t):
                                return spkp[:, t, :, :].rearrange(
                                    "p a b -> p (a b)"
                                )

                            nc.scalar.activation(memf, tsl(0), ACTF.Copy)
                            nc.vector.tensor_scalar(
                                ssl(0), memf, THRESH, None, op0=ALU.is_gt
                            )
                            # per-timestep spike writes (contiguous 256B-run
                            # planes) right after each compare: phase B's
                            # block (t,h) only waits on the LAST pair's
                            # t-slice
                            nc.scalar.dma_start(
                                spk1d[0, :, n - 1 : n + 1, :], spkp[:, 0, :, :]
                            )
                            for t in range(1, T):
                                nc.vector.scalar_tensor_tensor(
                                    memf, memf, BETA, ssl(t - 1),
                                    op0=ALU.mult, op1=ALU.subtract,
                                )
                                nc.vector.scalar_tensor_tensor(
                                    memf, memf, 1.0, tsl(t),
                                    op0=ALU.bypass, op1=ALU.add,
                                )
                                nc.vector.tensor_scalar(
                                    ssl(t), memf, THRESH, None, op0=ALU.is_gt
                                )
                                nc.scalar.dma_start(
                                    spk1d[t, :, n - 1 : n + 1, :],
                                    spkp[:, t, :, :],
                                )

                        # paced w2 gen: ~1.2 blocks per tile, tiles 1..26
                        for o2, tl in enumerate(w2tile):
                            if tl == n:
                                emit_w2gen(o2)

                        # gen compute for tile n+2 lands after this tile's LIF
                        # ops, so LIF never waits behind DMA-blocked gen ops
                        if n + 2 < NT:
                            pend[n + 2] = emit_w1gen_compute(n + 2)

                    # A->B handoff warm-keeper: fp32 dummy matmuls READING
                    # the last pair's currents - the data dependency pins
                    # them to the tail of phase A (dep-free dummies get
                    # hoisted to the start by the scheduler), and fp32 rate
                    # stretches ~10us of PE busy across the handoff so HAM
                    # never re-throttles; the junk ExternalOutput keeps the
                    # chain live
                    wps2 = wmps.tile([P, 512], F32)
                    kst = pcur[:, 0, 0, 0:128]
                    kmv = pcur[:, 1, :, :]
                    for i in range(24):
                        nc.tensor.matmul(
                            wps2, kst, kmv, start=(i == 0), stop=(i == 23)
                        )
                    nc.vector.tensor_copy(wmb[:, 256:264], wps2[:, 0:8])
                    nc.sync.dma_start(junk, wmb[:, 256:264])

                # ---------------- Phase B: layer 2 ----------------
                with (
                    tc.tile_pool(name="w2res2", bufs=1) as w2r2,
                    tc.tile_pool(name="bspk8", bufs=2) as bp8,
                    tc.tile_pool(name="bspk16", bufs=5) as bp16,
                    tc.tile_pool(name="bstate", bufs=2) as bs,
                    tc.tile_pool(name="bpsum", bufs=2, space="PSUM") as bps,
                ):
                    W2F2 = w2r2.tile([P, KO2, 512], F16)
                    # half-1 weights: data has been in DRAM since ~tile 26,
                    # only the SBUF WAR on phase A's space delays these; they
                    # fill during half 0's compute
                    for c in range(8):
                        o2s = slice(c * 4, (c + 1) * 4)
                        nc.gpsimd.dma_start(
                            W2F2[:, o2s, :],
                            w2h2d[o2s].rearrange("k p c -> p k c"),
                        )

                    spks_t = []
                    for half in range(2):
                        W2F = W2F1 if half == 0 else W2F2
                        acc = bs.tile([P, 2, 512], F32, tag="acc")
                        mem2 = bs.tile([P, 2, 512], F32, tag="mem2")
                        spk2 = bs.tile([P, 2, 512], F32, tag="spk2")
                        for t in range(T):
                            if half == 0:
                                spk8 = bp8.tile([P, NT, BC], F8, tag="spk8")
                                # split across sync+scalar so the load keeps
                                # ahead of the MM consume rate
                                nc.sync.dma_start(
                                    spk8[:, 0:16, :], spk1d[t, :, 0:16, :]
                                )
                                nc.scalar.dma_start(
                                    spk8[:, 16:NT, :], spk1d[t, :, 16:NT, :]
                                )
                                spk16 = bp16.tile([P, NT, BC], F16, tag="spk16")
                                # two k-half casts so the first MMs only wait
                                # ~2us, not the whole 8K-col upcast
                                for kh in range(2):
                                    ks = slice(kh * 16, (kh + 1) * 16)
                                    nc.vector.tensor_copy(
                                        spk16[:, ks, :].rearrange("p a b -> p (a b)"),
                                        spk8[:, ks, :].rearrange("p a b -> p (a b)"),
                                    )
                                spks_t.append(spk16)
                            else:
                                spk16 = spks_t[t]
                            for h in range(2):
                                csl = slice(h * P, (h + 1) * P)
                                ps2 = bps.tile([P, 512], F32, tag="ps2")
                                for k2 in range(KO2):
                                    nc.tensor.matmul(
                                        ps2, spk16[:, k2, csl], W2F[:, k2, :],
                                        start=(k2 == 0), stop=(k2 == KO2 - 1),
                                    )
                                m2 = mem2[:, h, :]
                                if t == 0:
                                    nc.scalar.activation(m2, ps2, ACTF.Copy)
                                    nc.vector.tensor_scalar(
                                        acc[:, h, :], m2, THRESH, None, op0=ALU.is_gt
                                    )
                                    nc.scalar.activation(
                                        spk2[:, h, :], acc[:, h, :], ACTF.Copy
                                    )
                                else:
                                    nc.vector.scalar_tensor_tensor(
                                        m2, m2, BETA, spk2[:, h, :],
                                        op0=ALU.mult, op1=ALU.subtract,
                                    )
                                    nc.vector.scalar_tensor_tensor(
                                        m2, m2, 1.0, ps2, op0=ALU.bypass, op1=ALU.add
                                    )
                                    if t < T - 1:
                                        nc.vector.tensor_scalar(
                                            spk2[:, h, :], m2, THRESH, None,
                                            op0=ALU.is_gt,
                                        )
                                    nc.vector.scalar_tensor_tensor(
                                        acc[:, h, :], m2, THRESH, acc[:, h, :],
                                        op0=ALU.is_gt, op1=ALU.add,
                                    )
                        osl = slice(half * 512, (half + 1) * 512)
                        for h in range(2):
                            nc.sync.dma_start(
                                out[h * P : (h + 1) * P, osl], acc[:, h, :]
                            )

    nc.compile()
    return nc


_NC_CACHE = None


def _get_nc():
    global _NC_CACHE
    if _NC_CACHE is None:
        _NC_CACHE = _build_nc()
    return _NC_CACHE


def _make_in_maps(inputs):
    x = np.ascontiguousarray(inputs["x"], dtype=np.float32)
    shared = {
        "w2_mu": np.ascontiguousarray(inputs["w2_mu"], dtype=np.float32),
        "w2_logvar": np.ascontiguousarray(inputs["w2_logvar"], dtype=np.float32),
        "eps2": np.ascontiguousarray(
            np.asarray(inputs["eps2"], dtype=np.float32).astype(np.float16)
        ),
    }
    for name in ("w1_mu", "w1_logvar", "eps1"):
        a = np.asarray(inputs[name], dtype=np.float32)
        # [ (o p), (n c) ] -> [p, n, o, c]: per-partition tile slices are
        # 8KB contiguous, so the gen loads run at full DMA rate
        shared[name] = np.ascontiguousarray(
            a.reshape(KO1, P, NT, P).transpose(1, 2, 0, 3)
        )
    in_maps = []
    for c in range(NCORES):
        xc = x[c * BC : (c + 1) * BC]          # [BC, T, DIN]
        xtc = np.ascontiguousarray(xc.transpose(2, 1, 0)).reshape(DIN, TB)
        # fp16 main plane + fp8 correction planes (RNE), matching the
        # on-chip weight split: xh8 = e4m3(x), xl8 = e4m3((x - xh16)*2^11)
        xh16 = xtc.astype(np.float16)
        xh8 = xtc.astype(ml_dtypes.float8_e4m3)
        xl8 = ((xtc - xh16.astype(np.float32)) * np.float32(WS)).astype(
            ml_dtypes.float8_e4m3
        )
        def tb_layout(a):
            # [ (o p), tb ] -> [p, o, tb]
            return np.ascontiguousarray(
                a.reshape(KO1, P, TB).transpose(1, 0, 2)
            )
        in_maps.append(
            {
                "xh16": tb_layout(xh16),
                "xh8": tb_layout(xh8),
                "xl8": tb_layout(xl8),
                **shared,
            }
        )
    return in_maps


def _run(inputs, trace=False, **kwargs):
    nc = _get_nc()
    in_maps = _make_in_maps(inputs)
    res = run_bass_kernel_spmd(
        nc, in_maps, core_ids=list(range(NCORES)), trace=trace, **kwargs
    )
    outs = [np.asarray(res.results[c]["out"]) for c in range(NCORES)]
    full = np.concatenate(outs, axis=0).astype(np.float32)
    return full, res


def kernel(**inputs):
    full, _ = _run(inputs, trace=False)
    return full


# revision 9
# speedup vs baseline: 1.0398x; 1.0398x over previous
"""Trainium2 Bass kernel for the Bayesian SNN problem.

Model (per reference):
  w1 = w1_mu + eps1 * exp(0.5*w1_logvar)          [2048, 4096]
  w2 = w2_mu + eps2 * exp(0.5*w2_logvar)          [4096, 1024]
  5-step LIF over batch 2048:
    mem = 0.95*mem + cur - (prev_mem > 1)
    spk = (mem > 1)
  out = sum_t spk2   -> [2048, 1024]

Strategy: pure data-parallel over batch (256 rows/core, 8 cores, no
collectives). The matmuls have no cross-timestep dependency, so each core
runs one batched GEMM over all (t, b) rows per layer; only the cheap
elementwise membrane scan is sequential in t.

GEMM1 runs as 2.13 passes instead of 3 bf16 passes: an fp16 main pass
(wh = fp16(w1), xh = fp16(x)) plus ONE fp8e4m3 DoubleRow correction pass
computing wl*xh + wh*xl with the residuals wl = w1-wh, xl = x-xh
pre-scaled by 2^11 so both products share one power-of-two psum scale
(main-pass weights also carry the 2^11 so everything accumulates in a
single psum group; the 2^-11 descale rides the psum-drain activation for
free). DoubleRow contracts 256 rows/instruction at fp8 rate, so the
correction pass costs ~1.13 K-passes -> per-tile matmul time drops from
25.6us to ~17.5us (HW-measured). Emulated numerics: 1879/2.1M output
flips, rel err 9.7e-3 (gate 2e-2); eps2 ships as fp16 (2012 flips), eps1
must stay f32 (fp16 eps1 -> 13k flips, over the gate).

GEMM2 is one fp16 pass split into two 512-wide output halves: half 0's
weights are generated directly into an SBUF tile that stays resident
through phase A (no DRAM round trip), half 1 goes through a DRAM fp16
scratch (a gpsimd cast-DMA, f32 SBUF -> f16 DRAM); the spike tensors
load once per timestep and stay resident across halves.

Spikes cross phases as fp8 (exact for {0,1}) in a t-major DRAM layout
[T, P, NT, BC] so phase B's loads are 256B-contiguous runs (the k-major
layout gathered 128B runs at ~140GB/s and fell behind the MM consume
rate). Phase A writes them per-(pair, timestep) so the A->B handoff only
waits on the LAST pair's t-slice, not a whole [P, TB] block; phase B
upcasts to fp16 on DVE (one [P, 8K] copy per timestep).

LIF in phase A runs on PAIRS of hidden tiles: the psum drains interleave
two tiles' currents into one [P, TB, 2] buffer so every LIF op covers 512
columns instead of 256 - halves the DVE op count, whose fixed ~0.3-0.4us
per-op cost otherwise backs up the strict-FIFO queues and (through the
lv/eps quarter-buffer WAR chain) stalls the w1-gen DMA stream.

Scheduling notes: an 80-MM dummy warmup holds the HAM clock-gate at K=8/8
through the pipeline fill (tiles 0/1 generate their weight splits at
quarter granularity so the first LDWEIGHTS can issue ~6us earlier); w1-gen
DMA issue is quarter-interleaved 2 tiles ahead with compute emitted after
the current tile's LIF (DVE/ACT queues are strict FIFO - order of emission
is everything); w2-gen is paced ~1.2 blocks per tile across tiles 1..26
with loads split scalar/gpsimd so no queue backs up behind the w1-gen
stream (sync); a 24-MM dummy chain at the top of phase B (kept live by an
ExternalOutput junk write - internal-scratch writes get dead-code
eliminated) bridges the pool-exit barrier so HAM never re-throttles at
the handoff.
"""

import ml_dtypes
import numpy as np

import concourse.bass as bass
import concourse.tile as tile
from concourse import bacc, mybir
from concourse.bass_utils import run_bass_kernel_spmd

F32 = mybir.dt.float32
BF16 = mybir.dt.bfloat16
F16 = mybir.dt.float16
F8 = mybir.dt.float8e4
ALU = mybir.AluOpType
ACTF = mybir.ActivationFunctionType
DR = mybir.MatmulPerfMode.DoubleRow

P = 128
B, T, DIN, DH, DOUT = 2048, 5, 2048, 4096, 1024
NCORES = 8
BC = B // NCORES            # 256 batch rows per core
TB = T * BC                 # 1280 batched-time rows per core
KO1 = DIN // P              # 16 contraction tiles, layer 1
NT = DH // P                # 32 hidden tiles
KO2 = DH // P               # 32 contraction tiles, layer 2
BETA = 0.95
THRESH = 1.0
WS = 2048.0                 # 2^11 residual / main-pass weight scale
DS = 1.0 / WS

MM_CHUNKS = ((0, 512), (512, 512), (1024, 256))


def _build_nc():
    nc = bacc.Bacc(
        "TRN2",
        target_bir_lowering=False,
        debug=False,
        num_devices=NCORES,
    )

    # x arrives host-split: fp16 main plane + fp8 e4m3 planes for the
    # DoubleRow correction (xh8 = e4m3(x), xl8 = e4m3((x - fp16(x))*2^11)),
    # all in [p, k_tile, tb] layout
    xh16_in = nc.dram_tensor("xh16", [P, KO1, TB], F16, kind="ExternalInput").ap()
    xh8_in = nc.dram_tensor("xh8", [P, KO1, TB], F8, kind="ExternalInput").ap()
    xl8_in = nc.dram_tensor("xl8", [P, KO1, TB], F8, kind="ExternalInput").ap()
    # w1 inputs come in host-pretransposed [p, n_tile, k_tile, col] layout:
    # each partition's (n) tile slice is 8KB contiguous -> full-rate DMA
    w1_mu = nc.dram_tensor("w1_mu", [P, NT, KO1, P], F32, kind="ExternalInput").ap()
    w1_lv = nc.dram_tensor("w1_logvar", [P, NT, KO1, P], F32, kind="ExternalInput").ap()
    eps1 = nc.dram_tensor("eps1", [P, NT, KO1, P], F32, kind="ExternalInput").ap()
    w2_mu = nc.dram_tensor("w2_mu", [DH, DOUT], F32, kind="ExternalInput").ap()
    w2_lv = nc.dram_tensor("w2_logvar", [DH, DOUT], F32, kind="ExternalInput").ap()
    eps2 = nc.dram_tensor("eps2", [DH, DOUT], F16, kind="ExternalInput").ap()
    out = nc.dram_tensor("out", [BC, DOUT], F32, kind="ExternalOutput").ap()
    # tiny real output keeping the warm-keeper chain alive through DCE
    junk = nc.dram_tensor("junk", [P, 8], BF16, kind="ExternalOutput").ap()

    with tile.TileContext(nc) as tc:
        with tc.tile_pool(name="dram", bufs=1, space="DRAM") as dramp:
            # t-major fp8 spikes: phase B's per-t load is 256B-contiguous
            spk1d = dramp.tile([T, P, NT, BC], F8)
            w2h2d = dramp.tile([KO2, P, 512], F16)

            # W2 half-0 residency and the warmup tiles live OUTSIDE the
            # phase scopes: w2-gen writes W2F1 during phase A and phase B
            # reads it; the warm tiles serve both the startup warmup and
            # the A->B handoff keeper.
            with (
                tc.tile_pool(name="w2res1", bufs=1) as w2r1,
                tc.tile_pool(name="warm", bufs=1) as wmp,
                tc.tile_pool(name="warmps", bufs=1, space="PSUM") as wmps,
            ):
                W2F1 = w2r1.tile([P, KO2, 512], F16)
                wma = wmp.tile([P, 128], BF16, tag="wma")
                wmb = wmp.tile([P, 512], BF16, tag="wmb")

                # ---------------- Phase A: layer 1 ----------------
                with (
                    tc.tile_pool(name="xres", bufs=1) as xp,
                    tc.tile_pool(name="aw1t", bufs=3) as wp1,
                    tc.tile_pool(name="aw1hl", bufs=3) as whl,
                    tc.tile_pool(name="aq", bufs=5) as qp,
                    tc.tile_pool(name="acur", bufs=1) as curp,
                    tc.tile_pool(name="aspk", bufs=1) as sp,
                    tc.tile_pool(name="amem", bufs=1) as mp,
                    tc.tile_pool(name="w2gen", bufs=1) as w2p,
                    tc.tile_pool(name="apsum", bufs=2, space="PSUM") as aps,
                ):
                    pend_data = {}

                    def emit_w1gen_loads(n):
                        """DMA mu/logvar/eps column-block for hidden tile n,
                        quarter-interleaved so the gen chain can start on
                        quarter 0 without waiting for the whole block."""
                        w1t = wp1.tile([P, KO1, P], F32, tag="w1t")
                        qs = []
                        for q in range(4):
                            osl = slice(q * 4, (q + 1) * 4)
                            lvq = qp.tile([P, 4, P], F32, tag="lv1")
                            epq = qp.tile([P, 4, P], F32, tag="ep1")
                            nc.sync.dma_start(lvq, w1_lv[:, n, osl, :])
                            nc.sync.dma_start(epq, eps1[:, n, osl, :])
                            nc.sync.dma_start(w1t[:, osl, :], w1_mu[:, n, osl, :])
                            qs.append((lvq, epq))
                        pend_data[n] = (w1t, qs)

                    def emit_w1gen_compute(n, fine=False):
                        """Build w1 = mu + eps*exp(0.5*lv) in f32, then split
                        into the fp16 main-pass weights (x 2^11) and the fp8
                        DoubleRow pair (wl*2^11 interleaved with wh).
                        Emitted a full tile after its loads so the DVE/ACT
                        queue heads never sit blocked on the gen DMAs.
                        fine=True (tiles 0/1) also splits the weight
                        conversions per quarter so the first matmuls'
                        LDWEIGHTS can issue before the whole tile is in."""
                        w1t, qs = pend_data.pop(n)
                        wh16 = whl.tile([P, KO1, P], F16, tag="wh16")
                        w18i = whl.tile([P, 2, KO1, P], F8, tag="w18i")

                        def convert(osl):
                            # flatten contiguous [P, a, P] -> [P, a*128]: 3D
                            # APs overflow the TT sync-wait encoding
                            w1f = w1t[:, osl, :].rearrange("p a b -> p (a b)")
                            wh16f = wh16[:, osl, :].rearrange("p a b -> p (a b)")
                            wl8f = w18i[:, 0, osl, :].rearrange("p a b -> p (a b)")
                            wh8f = w18i[:, 1, osl, :].rearrange("p a b -> p (a b)")
                            # wh16 = fp16(w1)*2^11 (exact power-of-2 scale)
                            nc.scalar.activation(wh16f, w1f, ACTF.Copy, scale=WS)
                            # wl8 = e4m3(w1*2^11 - wh16)  (= residual * 2^11)
                            nc.vector.scalar_tensor_tensor(
                                wl8f, w1f, WS, wh16f, op0=ALU.mult, op1=ALU.subtract
                            )
                            # wh8 = e4m3(w1), unscaled (pairs with xl8*2^11)
                            nc.scalar.activation(wh8f, w1f, ACTF.Copy)

                        for q, (lvq, epq) in enumerate(qs):
                            osl = slice(q * 4, (q + 1) * 4)
                            lvf = lvq.rearrange("p a b -> p (a b)")
                            epf = epq.rearrange("p a b -> p (a b)")
                            w1f = w1t[:, osl, :].rearrange("p a b -> p (a b)")
                            nc.scalar.activation(lvf, lvf, ACTF.Exp, scale=0.5)
                            nc.vector.scalar_tensor_tensor(
                                epf, epf, 1.0, lvf, op0=ALU.bypass, op1=ALU.mult
                            )
                            # split the adds across DVE/gpsimd (gpsimd runs
                            # these ~3x slower than spec; don't chain 4 there)
                            if q < 2:
                                nc.vector.scalar_tensor_tensor(
                                    w1f, w1f, 1.0, epf, op0=ALU.bypass, op1=ALU.add
                                )
                            else:
                                nc.gpsimd.tensor_tensor(w1f, w1f, epf, op=ALU.add)
                            if fine:
                                convert(osl)
                        if not fine:
                            convert(slice(0, KO1))
                        return wh16, w18i

                    # w2-gen pacing: one full k-row block per emission,
                    # ~1.2 per tile across tiles 1..26 so the gpsimd/DMA
                    # tail fully drains before the A->B handoff
                    w2tile = [1 + (o2 * 26) // KO2 for o2 in range(KO2)]

                    def emit_w2gen(o2):
                        """One k-row block of w2 -> fp16: cols 0:512 straight
                        into the resident W2F1 tile, cols 512: via DRAM
                        scratch (gpsimd cast-DMA). Loads split scalar/gpsimd
                        so neither queue backs up behind the w1-gen stream
                        (sync)."""
                        orows = slice(o2 * P, (o2 + 1) * P)
                        mu2t = w2p.tile([P, DOUT], F32, tag="mu2")
                        lv2t = w2p.tile([P, DOUT], F32, tag="lv2")
                        ep2t = w2p.tile([P, DOUT], F16, tag="ep2")
                        nc.gpsimd.dma_start(lv2t, w2_lv[orows, :])
                        nc.gpsimd.dma_start(ep2t, eps2[orows, :])
                        nc.scalar.dma_start(mu2t, w2_mu[orows, :])
                        nc.scalar.activation(lv2t, lv2t, ACTF.Exp, scale=0.5)
                        nc.vector.scalar_tensor_tensor(
                            lv2t, lv2t, 1.0, ep2t, op0=ALU.bypass, op1=ALU.mult
                        )
                        nc.gpsimd.tensor_tensor(mu2t, mu2t, lv2t, op=ALU.add)
                        nc.scalar.activation(W2F1[:, o2, :], mu2t[:, 0:512], ACTF.Copy)
                        nc.gpsimd.dma_start(w2h2d[o2], mu2t[:, 512:DOUT])

                    # PE warmup: ~17us of dummy matmuls so the HAM clock gate
                    # releases (K=8/8) before the real stream starts; otherwise
                    # the first ~10 tiles run cold and re-throttle in a cascade
                    nc.vector.memset(wma, 0.0)
                    nc.vector.memset(wmb, 0.0)
                    wps = wmps.tile([P, 512], F32)
                    for i in range(80):
                        nc.tensor.matmul(wps, wma, wmb, start=(i == 0), stop=(i == 79))
                    nc.vector.tensor_copy(wmb[:, 0:256], wps[:, 0:256])

                    # w-gen for the first two tiles comes first so the ACT/DVE
                    # queues aren't head-of-line blocked behind the x staging;
                    # the x chunks interleave with the two fine gen-computes
                    # (scheduler runs ready DMAs while converts wait on loads)
                    XH16 = xp.tile([P, KO1, TB], F16)
                    X8I = xp.tile([P, 2, KO1, TB], F8)
                    emit_w1gen_loads(0)
                    emit_w1gen_loads(1)
                    pend = {0: emit_w1gen_compute(0, fine=True)}
                    nc.scalar.dma_start(XH16[:, 0:2, :], xh16_in[:, 0:2, :])
                    nc.scalar.dma_start(XH16[:, 2:4, :], xh16_in[:, 2:4, :])
                    pend[1] = emit_w1gen_compute(1, fine=True)
                    for c in range(2, 8):
                        osl = slice(c * 2, (c + 1) * 2)
                        nc.scalar.dma_start(XH16[:, osl, :], xh16_in[:, osl, :])
                    for c in range(4):
                        osl = slice(c * 4, (c + 1) * 4)
                        nc.gpsimd.dma_start(X8I[:, 0, osl, :], xh8_in[:, osl, :])
                        nc.gpsimd.dma_start(X8I[:, 1, osl, :], xl8_in[:, osl, :])

                    pcur = None
                    for n in range(NT):
                        wh16, w18i = pend.pop(n)
                        if n + 2 < NT:
                            emit_w1gen_loads(n + 2)

                        ps = aps.tile([P, TB], F32, tag="ps1")
                        # fp16 main pass (psum holds 2^11 * cur1)
                        for k in range(KO1):
                            for c0, cw in MM_CHUNKS:
                                nc.tensor.matmul(
                                    ps[:, c0 : c0 + cw],
                                    wh16[:, k, :],
                                    XH16[:, k, c0 : c0 + cw],
                                    start=(k == 0),
                                    stop=False,
                                )
                        # fp8 DoubleRow correction pass: per k-tile one MM
                        # contracts wl*xh (slot 0) + wh*xl (slot 1)
                        for k in range(KO1):
                            for c0, cw in MM_CHUNKS:
                                nc.tensor.matmul(
                                    ps[:, c0 : c0 + cw],
                                    w18i[:, :, k, :],
                                    X8I[:, :, k, c0 : c0 + cw],
                                    start=False,
                                    stop=(k == KO1 - 1),
                                    perf_mode=DR,
                                )

                        # drain (with the 2^-11 descale) scatters this
                        # tile's currents into its parity plane of the pair
                        # buffer [P, T, 2, BC] (t-major so every LIF slice
                        # AND every spike-DMA slice is contiguous); the psum
                        # recycles fast and the serial LIF chain below never
                        # gates the tensor engine
                        if n % 2 == 0:
                            pcur = curp.tile([P, T, 2, BC], F32, tag="cur1")
                        nc.scalar.activation(
                            pcur[:, :, n % 2, :], ps, ACTF.Copy, scale=DS
                        )

                        if n % 2 == 1:
                            # pair-LIF over both tiles' currents: every op is
                            # [P, 512] instead of [P, 256] - half the DVE ops
                            spkp = sp.tile([P, T, 2, BC], F8, tag="spk")
                            mem = mp.tile([P, 2, BC], F32, tag="mem1")
                            memf = mem.rearrange("p a b -> p (a b)")

                            def tsl(t):
                                return pcur[:, t, :, :].rearrange(
                                    "p a b -> p (a b)"
                                )

                            def ssl(t):
                                return spkp[:, t, :, :].rearrange(
                                    "p a b -> p (a b)"
                                )

                            nc.scalar.activation(memf, tsl(0), ACTF.Copy)
                            nc.vector.tensor_scalar(
                                ssl(0), memf, THRESH, None, op0=ALU.is_gt
                            )
                            # per-timestep spike writes (contiguous 256B-run
                            # planes) right after each compare: phase B's
                            # block (t,h) only waits on the LAST pair's
                            # t-slice
                            nc.scalar.dma_start(
                                spk1d[0, :, n - 1 : n + 1, :], spkp[:, 0, :, :]
                            )
                            for t in range(1, T):
                                nc.vector.scalar_tensor_tensor(
                                    memf, memf, BETA, ssl(t - 1),
                                    op0=ALU.mult, op1=ALU.subtract,
                                )
                                nc.vector.scalar_tensor_tensor(
                                    memf, memf, 1.0, tsl(t),
                                    op0=ALU.bypass, op1=ALU.add,
                                )
                                nc.vector.tensor_scalar(
                                    ssl(t), memf, THRESH, None, op0=ALU.is_gt
                                )
                                nc.scalar.dma_start(
                                    spk1d[t, :, n - 1 : n + 1, :],
                                    spkp[:, t, :, :],
                                )

                        # paced w2 gen: ~1.2 blocks per tile, tiles 1..26
                        for o2, tl in enumerate(w2tile):
                            if tl == n:
                                emit_w2gen(o2)

                        # gen compute for tile n+2 lands after this tile's LIF
                        # ops, so LIF never waits behind DMA-blocked gen ops
                        if n + 2 < NT:
                            pend[n + 2] = emit_w1gen_compute(n + 2)

                    # A->B handoff warm-keeper: fp32 dummy matmuls READING
                    # the last pair's currents - the data dependency pins
                    # them to the tail of phase A (dep-free dummies get
                    # hoisted to the start by the scheduler), and fp32 rate
                    # stretches ~10us of PE busy across the handoff so HAM
                    # never re-throttles; the junk ExternalOutput keeps the
                    # chain live
                    wps2 = wmps.tile([P, 512], F32)
                    kst = pcur[:, 0, 0, 0:128]
                    kmv = pcur[:, 1, :, :]
                    for i in range(8):
                        nc.tensor.matmul(
                            wps2, kst, kmv, start=(i == 0), stop=(i == 7)
                        )
                    nc.vector.tensor_copy(wmb[:, 256:264], wps2[:, 0:8])
                    nc.sync.dma_start(junk, wmb[:, 256:264])

                # ---------------- Phase B: layer 2 ----------------
                with (
                    tc.tile_pool(name="w2res2", bufs=1) as w2r2,
                    tc.tile_pool(name="bspk8", bufs=2) as bp8,
                    tc.tile_pool(name="bspk16", bufs=5) as bp16,
                    tc.tile_pool(name="bstate", bufs=2) as bs,
                    tc.tile_pool(name="bpsum", bufs=2, space="PSUM") as bps,
                ):
                    W2F2 = w2r2.tile([P, KO2, 512], F16)
                    # half-1 weights: data has been in DRAM since ~tile 26,
                    # only the SBUF WAR on phase A's space delays these; they
                    # fill during half 0's compute
                    for c in range(8):
                        o2s = slice(c * 4, (c + 1) * 4)
                        nc.gpsimd.dma_start(
                            W2F2[:, o2s, :],
                            w2h2d[o2s].rearrange("k p c -> p k c"),
                        )

                    spks_t = []
                    for half in range(2):
                        W2F = W2F1 if half == 0 else W2F2
                        acc = bs.tile([P, 2, 512], F32, tag="acc")
                        mem2 = bs.tile([P, 2, 512], F32, tag="mem2")
                        spk2 = bs.tile([P, 2, 512], F32, tag="spk2")
                        for t in range(T):
                            if half == 0:
                                spk8 = bp8.tile([P, NT, BC], F8, tag="spk8")
                                # split across sync+scalar so the load keeps
                                # ahead of the MM consume rate
                                nc.sync.dma_start(
                                    spk8[:, 0:16, :], spk1d[t, :, 0:16, :]
                                )
                                nc.scalar.dma_start(
                                    spk8[:, 16:NT, :], spk1d[t, :, 16:NT, :]
                                )
                                spk16 = bp16.tile([P, NT, BC], F16, tag="spk16")
                                # two k-half casts so the first MMs only wait
                                # ~2us, not the whole 8K-col upcast
                                for kh in range(2):
                                    ks = slice(kh * 16, (kh + 1) * 16)
                                    nc.vector.tensor_copy(
                                        spk16[:, ks, :].rearrange("p a b -> p (a b)"),
                                        spk8[:, ks, :].rearrange("p a b -> p (a b)"),
                                    )
                                spks_t.append(spk16)
                            else:
                                spk16 = spks_t[t]
                            for h in range(2):
                                csl = slice(h * P, (h + 1) * P)
                                ps2 = bps.tile([P, 512], F32, tag="ps2")
                                for k2 in range(KO2):
                                    nc.tensor.matmul(
                                        ps2, spk16[:, k2, csl], W2F[:, k2, :],
                                        start=(k2 == 0), stop=(k2 == KO2 - 1),
                                    )
                                m2 = mem2[:, h, :]
                                if t == 0:
                                    nc.scalar.activation(m2, ps2, ACTF.Copy)
                                    nc.vector.tensor_scalar(
                                        acc[:, h, :], m2, THRESH, None, op0=ALU.is_gt
                                    )
                                    nc.scalar.activation(
                                        spk2[:, h, :], acc[:, h, :], ACTF.Copy
                                    )
                                else:
                                    nc.vector.scalar_tensor_tensor(
                                        m2, m2, BETA, spk2[:, h, :],
                                        op0=ALU.mult, op1=ALU.subtract,
                                    )
                                    nc.vector.scalar_tensor_tensor(
                                        m2, m2, 1.0, ps2, op0=ALU.bypass, op1=ALU.add
                                    )
                                    if t < T - 1:
                                        nc.vector.tensor_scalar(
                                            spk2[:, h, :], m2, THRESH, None,
                                            op0=ALU.is_gt,
                                        )
                                    nc.vector.scalar_tensor_tensor(
                                        acc[:, h, :], m2, THRESH, acc[:, h, :],
                                        op0=ALU.is_gt, op1=ALU.add,
                                    )
                        osl = slice(half * 512, (half + 1) * 512)
                        for h in range(2):
                            nc.sync.dma_start(
                                out[h * P : (h + 1) * P, osl], acc[:, h, :]
                            )

    nc.compile()
    return nc


_NC_CACHE = None


def _get_nc():
    global _NC_CACHE
    if _NC_CACHE is None:
        _NC_CACHE = _build_nc()
    return _NC_CACHE


def _make_in_maps(inputs):
    x = np.ascontiguousarray(inputs["x"], dtype=np.float32)
    shared = {
        "w2_mu": np.ascontiguousarray(inputs["w2_mu"], dtype=np.float32),
        "w2_logvar": np.ascontiguousarray(inputs["w2_logvar"], dtype=np.float32),
        "eps2": np.ascontiguousarray(
            np.asarray(inputs["eps2"], dtype=np.float32).astype(np.float16)
        ),
    }
    for name in ("w1_mu", "w1_logvar", "eps1"):
        a = np.asarray(inputs[name], dtype=np.float32)
        # [ (o p), (n c) ] -> [p, n, o, c]: per-partition tile slices are
        # 8KB contiguous, so the gen loads run at full DMA rate
        shared[name] = np.ascontiguousarray(
            a.reshape(KO1, P, NT, P).transpose(1, 2, 0, 3)
        )
    in_maps = []
    for c in range(NCORES):
        xc = x[c * BC : (c + 1) * BC]          # [BC, T, DIN]
        xtc = np.ascontiguousarray(xc.transpose(2, 1, 0)).reshape(DIN, TB)
        # fp16 main plane + fp8 correction planes (RNE), matching the
        # on-chip weight split: xh8 = e4m3(x), xl8 = e4m3((x - xh16)*2^11)
        xh16 = xtc.astype(np.float16)
        xh8 = xtc.astype(ml_dtypes.float8_e4m3)
        xl8 = ((xtc - xh16.astype(np.float32)) * np.float32(WS)).astype(
            ml_dtypes.float8_e4m3
        )
        def tb_layout(a):
            # [ (o p), tb ] -> [p, o, tb]
            return np.ascontiguousarray(
                a.reshape(KO1, P, TB).transpose(1, 0, 2)
            )
        in_maps.append(
            {
                "xh16": tb_layout(xh16),
                "xh8": tb_layout(xh8),
                "xl8": tb_layout(xl8),
                **shared,
            }
        )
    return in_maps


def _run(inputs, trace=False, **kwargs):
    nc = _get_nc()
    in_maps = _make_in_maps(inputs)
    res = run_bass_kernel_spmd(
        nc, in_maps, core_ids=list(range(NCORES)), trace=trace, **kwargs
    )
    outs = [np.asarray(res.results[c]["out"]) for c in range(NCORES)]
    full = np.concatenate(outs, axis=0).astype(np.float32)
    return full, res


def kernel(**inputs):
    full, _ = _run(inputs, trace=False)
    return full
